# revision 1
# baseline (speedup 1.0000x reference)
"""Trainium2 Bass kernel for a GroupNorm + cross-attention block.

Reference computation (shapes hardcoded):
  x:[2,512,16,16,16] -> GroupNorm(32 groups over (16ch x 4096 spatial))
  q = xn_seq @ Wq ; k,v = context @ Wk/Wv  (context:[2,1024,768])
  attn = softmax(q k^T / 8) ; out = (attn v) @ Wo + bo + residual
  output: [2,512,16,16,16]

Sharding: 8 cores = 2 batches x 4 sequence-quarters (1024 of 4096 voxel
tokens each). Each core computes all 8 heads for its token slice; the only
cross-core communication is a [8,8] f32 AllReduce of GroupNorm statistics
within each 4-core batch group.

Schedule notes (v2):
 - One DMA per input tensor (host pre-packs a [128, *] partition-major
   layout) so the SP engine isn't serialized issuing ~30 small DMAs.
 - GroupNorm is folded into the q projection (Wq rows scaled by
   A = gamma*rstd plus a rank-1 offset q0 = Wq^T B).
 - Attention is software-pipelined over 8 (token-chunk, head-pair) units:
   unit u's score matmuls + exp interleave with unit u-1's AV matmuls, so
   the activation engine (the attention-phase bottleneck: 64 exp tiles)
   never starves while the PE stays warm. K-pair/V projections that are
   not needed until later units are stuffed into the early units' PE gaps.
 - Softmax denominator comes from an all-ones column appended to V
   (row 64 of the AV matmul), reciprocals batched per unit, broadcast
   across partitions via a DRAM bounce, applied by DVE while drains of
   PSUM AV happen in the same op.
 - PSUM: 2x [128,1024] score tiles (4 banks) + a shared 4-slot 1-bank
   pool for every other matmul target (projections, AV, Wo).
"""

import os
from contextlib import ExitStack, nullcontext

import numpy as np

import concourse.bass as bass
import concourse.mybir as mybir
import concourse.tile as tile
from concourse import bacc, bass_utils

F32 = mybir.dt.float32
F32R = mybir.dt.float32r
BF16 = mybir.dt.bfloat16
AF = mybir.ActivationFunctionType
ALU = mybir.AluOpType

B = 2
C = 512
N = 4096            # voxel tokens per batch (16*16*16)
NL = 1024           # tokens per core (N / 4)
CTX = 1024
CTXD = 768
HEADS = 8
HD = 64
INNER = HEADS * HD  # 512
GROUPS = 32
EPS = 1e-5
SCALE = HD ** -0.5

CT = C // 128       # 4 channel tiles
KT = CTXD // 128    # 6 context-dim tiles
MT = CTX // 128     # 8 ctx row tiles
PAIRS = HEADS // 2  # 4 head-pair tiles (128 rows each)

# attention unit order: (ncc token-chunk, j head-pair). Ordered so kT[j]
# is needed two units after kT[j-1] (slack for interleaved K projection)
# and the ncc=0 units finish early enough that Wo(ncc=0) overlaps the
# remaining ncc=1 units.
UNITS = [(0, 0), (1, 0), (0, 1), (1, 1), (0, 2), (0, 3), (1, 2), (1, 3)]

_CACHED_NC = None


def build_nc(loop_iters=1, skip_collective=False):
    # Bacc (not raw Bass): its finalize() runs the wait-splitting passes
    # (move_matmul_waits_to_ldweights / generate_event_semaphores) that the
    # TRN2 ISA requires — walrus rejects multi-wait matmuls otherwise.
    nc = bacc.Bacc("TRN2", target_bir_lowering=False, debug=False, num_devices=8)

    # ctx / Wk / Wv / Wo travel as bf16: they only feed matmuls whose other
    # operand is already bf16 (K^T, V, scores, AV, Wo), and halving their
    # bytes cuts the DMA-bound load phase. x and Wq stay f32 (GroupNorm
    # stats, residual, and the f32r q projection need them).
    x_d = nc.dram_tensor("x_l", [128, CT * NL], F32R, kind="ExternalInput")
    ctxT_d = nc.dram_tensor("ctxT", [128, KT * CTX], BF16, kind="ExternalInput")
    wq_d = nc.dram_tensor("wq", [128, CT * INNER], F32R, kind="ExternalInput")
    wk_d = nc.dram_tensor("wk", [128, KT * INNER], BF16, kind="ExternalInput")
    wv_d = nc.dram_tensor("wv", [128, KT * INNER], BF16, kind="ExternalInput")
    wo_d = nc.dram_tensor("wo", [128, PAIRS * C], BF16, kind="ExternalInput")
    # packed consts: cols 0:8 sel_fwd | 8:12 gamma | 12:16 beta | 16:20 bo
    cst_d = nc.dram_tensor("cst", [128, 8 + 3 * CT], F32, kind="ExternalInput")
    sel_bwd_d = nc.dram_tensor("sel_bwd", [8, 128], F32, kind="ExternalInput")
    out_d = nc.dram_tensor("out_l", [128, CT, NL], F32, kind="ExternalOutput")

    stats_in_d = nc.dram_tensor("stats_in", [8, 8], F32)
    stats_out_d = nc.dram_tensor("stats_out", [8, 8], F32)
    # scratch for partition-broadcasting softmax denominators (DRAM bounce:
    # SBUF/PSUM sources cannot be read with partition-stride 0, DRAM can)
    den_d = nc.dram_tensor("den_scratch", [8, 2, 512], F32)

    with tile.TileContext(nc) as tc, ExitStack() as ctx:
        consts = ctx.enter_context(tc.tile_pool(name="consts", bufs=1))
        xpool = ctx.enter_context(tc.tile_pool(name="x", bufs=1))
        cxpool = ctx.enter_context(tc.tile_pool(name="ctx", bufs=1))
        wpool = ctx.enter_context(tc.tile_pool(name="weights", bufs=1))
        ktpool = ctx.enter_context(tc.tile_pool(name="kt", bufs=1))
        vpool = ctx.enter_context(tc.tile_pool(name="v", bufs=1))
        qpool = ctx.enter_context(tc.tile_pool(name="qt", bufs=1))
        epool = ctx.enter_context(tc.tile_pool(name="e", bufs=16))
        otpool = ctx.enter_context(tc.tile_pool(name="ot", bufs=1))
        spool = ctx.enter_context(tc.tile_pool(name="small", bufs=4))
        dpool = ctx.enter_context(tc.tile_pool(name="den", bufs=2))
        opool = ctx.enter_context(tc.tile_pool(name="outs", bufs=2))

        pst = ctx.enter_context(tc.tile_pool(name="pst", bufs=2, space="PSUM"))
        pav = ctx.enter_context(tc.tile_pool(name="pav", bufs=4, space="PSUM"))

        loop_cm = tc.For_i(0, loop_iters, 1) if loop_iters > 1 else nullcontext()
        with loop_cm:

            # ---- constant loads ----
            cst_sb = consts.tile([128, 8 + 3 * CT], F32, tag="cst")
            nc.sync.dma_start(out=cst_sb, in_=cst_d[:, :])
            sel_bwd = consts.tile([8, 128], F32, tag="sel_bwd")
            nc.sync.dma_start(out=sel_bwd, in_=sel_bwd_d[:, :])
            eps_sb = consts.tile([128, 1], F32, tag="eps")
            nc.vector.memset(eps_sb, EPS)
            sel_fwd = cst_sb[:, 0:8]
            gamma_sb = cst_sb[:, 8:8 + CT]
            beta_sb = cst_sb[:, 12:12 + CT]
            bo_sb = cst_sb[:, 16:16 + CT]

            # V tiles: ones column (index HD of each head's 65-wide block)
            # written once per iteration on the idle gpsimd engine.
            v_sb = []
            for m in range(MT):
                vt = vpool.tile([128, HEADS, HD + 1], BF16, tag=f"v{m}", name=f"vt{m}")
                nc.vector.memset(vt[:, :, HD:HD + 1], 1.0)
                v_sb.append(vt)

            # ---- bulk input loads (one DMA each; issue order = priority).
            # x is split per channel-tile so GroupNorm stats start while the
            # rest streams in; the tiny stats AllReduce bounce is issued
            # before the remaining bulk loads so it doesn't queue behind them.
            x_all = xpool.tile([128, CT * NL], F32R, tag="x")
            for t in range(CT):
                nc.sync.dma_start(
                    out=x_all[:, t * NL:(t + 1) * NL],
                    in_=x_d[:, t * NL:(t + 1) * NL],
                )
            wq_all = wpool.tile([128, CT * INNER], F32R, tag="wq")
            for h2 in range(2):
                hw_q = CT * INNER // 2
                nc.sync.dma_start(
                    out=wq_all[:, h2 * hw_q:(h2 + 1) * hw_q],
                    in_=wq_d[:, h2 * hw_q:(h2 + 1) * hw_q],
                )
            wk_all = wpool.tile([128, KT * INNER], BF16, tag="wk")
            ctx_all = cxpool.tile([128, KT * CTX], BF16, tag="ctx")
            wv_all = wpool.tile([128, KT * INNER], BF16, tag="wv")
            wo_all = wpool.tile([128, PAIRS * C], BF16, tag="wo")

            def xs(t, lo, hi):
                return x_all[:, t * NL + lo: t * NL + hi]

            def cxs(kk, lo, hi):
                return ctx_all[:, kk * CTX + lo: kk * CTX + hi]

            def wqs(t, lo, hi):
                return wq_all[:, t * INNER + lo: t * INNER + hi]

            def wks(kk, lo, hi):
                return wk_all[:, kk * INNER + lo: kk * INNER + hi]

            def wvs(kk):
                return wv_all[:, kk * INNER:(kk + 1) * INNER]

            def wos(jj, lo, hi):
                return wo_all[:, jj * C + lo: jj * C + hi]

            # ---- GroupNorm statistics ----
            # per-channel (mean, E[x^2]) over the local token slice, group-reduced
            # on the PE with sel_fwd (value 1/64: 16 channels x 4 cores), then
            # AllReduced within the batch group.
            ps_stats = pav.tile([128, 512], F32, tag="av", name="ps_stats")
            for t in range(CT):
                st6 = spool.tile([128, 2, 6], F32, tag="bn6")
                for sg in range(2):
                    nc.vector.bn_stats(
                        out=st6[:, sg, :], in_=xs(t, sg * 512, (sg + 1) * 512)
                    )
                mv = spool.tile([128, 2], F32, tag="mv")
                nc.vector.bn_aggr(out=mv, in_=st6)
                s12 = spool.tile([128, 2], F32, tag="s12")
                nc.vector.tensor_copy(out=s12[:, 0:1], in_=mv[:, 0:1])
                nc.vector.tensor_mul(s12[:, 1:2], mv[:, 0:1], mv[:, 0:1])
                nc.vector.tensor_add(s12[:, 1:2], s12[:, 1:2], mv[:, 1:2])
                nc.tensor.matmul(
                    ps_stats[0:8, t * 2:t * 2 + 2], lhsT=sel_fwd, rhs=s12,
                    start=True, stop=True,
                )
            stats_sb = spool.tile([8, 8], F32, tag="gst")
            nc.vector.tensor_copy(out=stats_sb, in_=ps_stats[0:8, 0:8])
            nc.sync.dma_start(out=stats_in_d[:, :], in_=stats_sb)
            if skip_collective:
                # timing-only variant: collectives inside a device-side For_i
                # desync the mesh on the 2nd iteration, so the timing loop
                # substitutes a local DRAM copy (output values are wrong by a
                # constant stats factor; latency profile is comparable).
                nc.sync.dma_start(out=stats_out_d[:, :], in_=stats_in_d[:, :])
            else:
                nc.gpsimd.collective_compute(
                    "AllReduce",
                    ALU.add,
                    replica_groups=[[0, 1, 2, 3], [4, 5, 6, 7]],
                    ins=[stats_in_d[:, :]],
                    outs=[stats_out_d[:, :]],
                )
            g_sb = spool.tile([8, 8], F32, tag="gout")
            nc.sync.dma_start(out=g_sb, in_=stats_out_d[:, :])

            # remaining bulk loads, chunked so the tiny stats-bounce DMAs
            # above never queue behind a multi-us transfer
            for h2 in range(2):
                hw_k = KT * INNER // 2
                nc.sync.dma_start(
                    out=wk_all[:, h2 * hw_k:(h2 + 1) * hw_k],
                    in_=wk_d[:, h2 * hw_k:(h2 + 1) * hw_k],
                )
            sixth_c = KT * CTX // 6
            for cc6 in range(6):
                nc.sync.dma_start(
                    out=ctx_all[:, cc6 * sixth_c:(cc6 + 1) * sixth_c],
                    in_=ctxT_d[:, cc6 * sixth_c:(cc6 + 1) * sixth_c],
                )
            for h2 in range(2):
                hw_v = KT * INNER // 2
                nc.sync.dma_start(
                    out=wv_all[:, h2 * hw_v:(h2 + 1) * hw_v],
                    in_=wv_d[:, h2 * hw_v:(h2 + 1) * hw_v],
                )
            nc.sync.dma_start(out=wo_all, in_=wo_d[:, :])

            # Per channel-tile affine coefficients:
            #   A = gamma * rstd,  B = beta - mean*rstd*gamma
            # rstd = exp(-0.5 * ln(var + eps)) keeps everything in the single
            # natural_log_exp activation table set shared with the softmax exp.
            cA, cB = [], []
            for t in range(CT):
                m1 = g_sb[:, 2 * t:2 * t + 1]
                m2 = g_sb[:, 2 * t + 1:2 * t + 2]
                var8 = spool.tile([8, 1], F32, tag=f"var{t}")
                nc.vector.tensor_mul(var8, m1, m1)
                nc.vector.tensor_sub(var8, m2, var8)
                nc.scalar.activation(var8, var8, AF.Ln, bias=eps_sb[0:8, :])
                nc.scalar.activation(var8, var8, AF.Exp, scale=-0.5)  # rstd
                ab8 = spool.tile([8, 2], F32, tag=f"ab{t}")
                nc.vector.tensor_copy(out=ab8[:, 0:1], in_=var8)
                nc.vector.tensor_mul(ab8[:, 1:2], m1, var8)  # mean*rstd
                ps_ab = pav.tile([128, 2], F32, tag="av", name="ps_ab")
                nc.tensor.matmul(
                    ps_ab[:, 0:2], lhsT=sel_bwd, rhs=ab8, start=True, stop=True
                )
                a_t = spool.tile([128, 1], F32, tag=f"cA{t}")
                b_t = spool.tile([128, 1], F32, tag=f"cB{t}")
                nc.vector.tensor_mul(a_t, ps_ab[:, 0:1], gamma_sb[:, t:t + 1])
                nc.vector.tensor_mul(b_t, ps_ab[:, 1:2], gamma_sb[:, t:t + 1])
                nc.vector.tensor_sub(b_t, beta_sb[:, t:t + 1], b_t)
                cA.append(a_t)
                cB.append(b_t)

            # q0 = Wq^T B per head-pair tile (before Wq is scaled in place).
            q0_sb = []
            for j in range(PAIRS):
                pq0 = pav.tile([128, 1], F32, tag="av", name="pq0")
                for t in range(CT):
                    nc.tensor.matmul(
                        pq0[:, 0:1],
                        lhsT=wqs(t, j * 128, (j + 1) * 128).bitcast(F32),
                        rhs=cB[t],
                        start=(t == 0), stop=(t == CT - 1),
                    )
                q0 = spool.tile([128, 1], F32, tag=f"q0{j}")
                nc.vector.tensor_copy(out=q0, in_=pq0[:, 0:1])
                q0_sb.append(q0)
            # Fold A into Wq rows in place (q0 above already consumed raw Wq).
            # Split across DVE and the idle gpsimd so the serial latency on
            # the collective->q critical path is halved.
            for t in range(CT):
                nc.vector.tensor_scalar_mul(
                    out=wq_all[:, t * INNER:(t + 1) * INNER],
                    in0=wq_all[:, t * INNER:(t + 1) * INNER],
                    scalar1=cA[t],
                )

            # ---- q^T (per head-pair tile), GroupNorm pre-folded ----
            # pairs 0/1 up front (units 0-3); pairs 2/3 deferred into the
            # attention loop's PE gaps.
            qT_sb = [
                qpool.tile([128, NL], BF16, tag=f"qT{j}", name=f"qt{j}")
                for j in range(PAIRS)
            ]

            def emit_qpair(j):
                for ncc in range(2):
                    pq = pav.tile([128, 512], F32, tag="av", name="pq")
                    for t in range(CT):
                        nc.tensor.matmul(
                            pq,
                            lhsT=wqs(t, j * 128, (j + 1) * 128),
                            rhs=xs(t, ncc * 512, (ncc + 1) * 512),
                            start=(t == 0), stop=(t == CT - 1),
                        )
                    nc.vector.tensor_scalar_add(
                        out=qT_sb[j][:, ncc * 512:(ncc + 1) * 512],
                        in0=pq,
                        scalar1=q0_sb[j],
                    )

            emit_qpair(0)
            emit_qpair(1)

            # ---- deferred projections, interleaved into the attention loop ----
            kT_sb = [
                ktpool.tile([128, CTX], BF16, tag=f"kT{j}", name=f"kt{j}")
                for j in range(PAIRS)
            ]

            def emit_khalf(j, cc):
                # K^T = (context @ Wk)^T for head-pair j, 512-col half cc;
                # PSUM drained by ACT (idle before the exp stream ramps).
                pk = pav.tile([128, 512], F32, tag="av", name="pk")
                for kk in range(KT):
                    nc.tensor.matmul(
                        pk,
                        lhsT=wks(kk, j * 128, (j + 1) * 128),
                        rhs=cxs(kk, cc * 512, (cc + 1) * 512),
                        start=(kk == 0), stop=(kk == KT - 1),
                    )
                nc.scalar.activation(
                    kT_sb[j][:, cc * 512:(cc + 1) * 512], pk, AF.Copy
                )

            def emit_vtile(m):
                # V = context @ Wv for ctx row-tile m (all heads); one strided
                # DVE copy drops the [128,512] PSUM into the 65-stride layout.
                pv = pav.tile([128, 512], F32, tag="av", name="pv")
                for kk in range(KT):
                    nc.tensor.matmul(
                        pv,
                        lhsT=cxs(kk, m * 128, (m + 1) * 128),
                        rhs=wvs(kk),
                        start=(kk == 0), stop=(kk == KT - 1),
                    )
                nc.vector.tensor_copy(out=v_sb[m][:, :, 0:HD], in_=pv)

            emit_khalf(0, 0)
            emit_khalf(0, 1)

            # ---- software-pipelined attention ----
            # per unit u: scores+exp for u interleaved with AV for u-1.
            ot_sb = [
                otpool.tile([128, NL], BF16, tag=f"ot{j}", name=f"ot{j}")
                for j in range(PAIRS)
            ]
            e_live = {}
            avp = {}
            # extra PE work stuffed into each unit's exp-paced gaps. kT[j]/
            # qT[j] are first consumed at unit index 2j (j=3 at index 5); V
            # tiles at unit 1 (AV of unit 0 consumes vt[m] at its m-th step).
            filler = {
                0: [("K", 1, 0), ("K", 1, 1), ("V", 0, 0), ("V", 1, 0)],
                1: [("V", m, 0) for m in range(2, MT)],
                2: [("K", 2, 0), ("Q", 2, 0)],
                3: [("K", 2, 1), ("Q", 3, 0)],
                4: [("K", 3, 0), ("K", 3, 1)],
            }

            def emit_av_m(u, m):
                ncc, j = UNITS[u]
                for half in range(2):
                    h = 2 * j + half
                    es = slice(half * 512, (half + 1) * 512)
                    nc.tensor.matmul(
                        avp[u][half],
                        lhsT=v_sb[m][:, h, :],
                        rhs=e_live[u][m][:, es],
                        start=(m == 0), stop=(m == MT - 1),
                    )

            def emit_den(u):
                # reciprocal of the ones-row sums (DVE), broadcast across the
                # 64 head-dim partitions on the idle gpsimd engine, then
                # normalize+drain AV PSUM into ot in one DVE op per half.
                ncc, j = UNITS[u]
                cs = slice(ncc * 512, (ncc + 1) * 512)
                for half in range(2):
                    rs = slice(half * HD, (half + 1) * HD)
                    denh = dpool.tile([1, 512], F32, tag="denh", bufs=4, name="denh")
                    nc.vector.reciprocal(
                        out=denh, in_=avp[u][half][HD:HD + 1, :]
                    )
                    nc.sync.dma_start(out=den_d[u, half], in_=denh)
                    drow = den_d[u, half]
                    bc_ap = bass.AP(
                        tensor=drow.tensor,
                        offset=drow.offset,
                        ap=[[0, HD], [1, 512]],
                    )
                    rec = dpool.tile([HD, 512], F32, tag="rec", bufs=4, name="rec")
                    nc.sync.dma_start(out=rec, in_=bc_ap)
                    nc.vector.tensor_mul(
                        ot_sb[j][rs, cs], avp[u][half][0:HD, :], rec
                    )

            def emit_wo(ncc):
                # out = OT^T Wo + bo + residual for this token chunk.
                cs = slice(ncc * 512, (ncc + 1) * 512)
                res = opool.tile([128, CT * 512], F32, tag="res", name="res")
                for t in range(CT):
                    po = pav.tile([128, 512], F32, tag="av", name="po")
                    for jj in range(PAIRS):
                        nc.tensor.matmul(
                            po,
                            lhsT=wos(jj, t * 128, (t + 1) * 128),
                            rhs=ot_sb[jj][:, cs],
                            start=(jj == 0), stop=(jj == PAIRS - 1),
                        )
                    nc.vector.scalar_tensor_tensor(
                        out=res[:, t * 512:(t + 1) * 512],
                        in0=po,
                        scalar=bo_sb[:, t:t + 1],
                        in1=xs(t, ncc * 512, (ncc + 1) * 512),
                        op0=ALU.add,
                        op1=ALU.add,
                    )
                    # per-tile store so the tail DMA overlaps the next STT
                    nc.sync.dma_start(
                        out=out_d[:, t, cs], in_=res[:, t * 512:(t + 1) * 512]
                    )

            for u in range(len(UNITS) + 1):
                if u < len(UNITS):
                    ncc, j = UNITS[u]
                    cs = slice(ncc * 512, (ncc + 1) * 512)
                    last = u == len(UNITS) - 1
                    if u > 0:
                        avp[u - 1] = [
                            pav.tile([HD + 1, 512], F32, tag="av", name="avp"),
                            pav.tile([HD + 1, 512], F32, tag="av", name="avp"),
                        ]
                    if last:
                        avp[u] = [
                            pav.tile([HD + 1, 512], F32, tag="av", name="avp"),
                            pav.tile([HD + 1, 512], F32, tag="av", name="avp"),
                        ]
                    fill = filler.get(u, [])
                    e_list = []
                    e_live[u] = e_list
                    for m in range(MT):
                        stp = pst.tile([128, 1024], F32, tag="st", name="stp")
                        nc.tensor.matmul(
                            stp[:, 0:512],
                            lhsT=kT_sb[j][0:HD, m * 128:(m + 1) * 128],
                            rhs=qT_sb[j][0:HD, cs],
                            start=True, stop=True,
                        )
                        nc.tensor.matmul(
                            stp[:, 512:1024],
                            lhsT=kT_sb[j][HD:128, m * 128:(m + 1) * 128],
                            rhs=qT_sb[j][HD:128, cs],
                            start=True, stop=True,
                        )
                        et = epool.tile([128, 1024], BF16, tag="e", name="et")
                        nc.scalar.activation(et, stp, AF.Exp, scale=SCALE)
                        e_list.append(et)
                        if fill:
                            kind, idx, cc = fill.pop(0)
                            if kind == "K":
                                emit_khalf(idx, cc)
                            elif kind == "Q":
                                emit_qpair(idx)
                            else:
                                emit_vtile(idx)
                        if u > 0:
                            emit_av_m(u - 1, m)
                        if last and m > 0:
                            # shrink the tail: the final unit's AV trails its
                            # own exp stream by one ctx tile
                            emit_av_m(u, m - 1)
                    if u > 0:
                        emit_den(u - 1)
                else:
                    # epilogue: finish the final unit
                    emit_av_m(u - 1, MT - 1)
                    emit_den(u - 1)
                if u == 6:
                    emit_wo(0)  # ncc=0 units (0,2,4,5) all normalized by now
            emit_wo(1)

    nc.finalize()
    return nc


def _host_prep(x, context, gamma, beta, Wq, Wk, Wv, Wo, bo):
    """Build the 8 per-core input maps (host-side slicing/transposes only)."""
    x2 = np.ascontiguousarray(x, np.float32).reshape(B, C, N)
    ctx = np.ascontiguousarray(context, np.float32)

    sel_fwd = np.zeros((128, 8), np.float32)
    for p in range(128):
        sel_fwd[p, p // 16] = 1.0 / 64.0  # 16 channels x 4 cores
    sel_bwd = np.zeros((8, 128), np.float32)
    for p in range(128):
        sel_bwd[p // 16, p] = 1.0

    cst = np.concatenate(
        [
            sel_fwd,
            np.asarray(gamma, np.float32).reshape(CT, 128).T,
            np.asarray(beta, np.float32).reshape(CT, 128).T,
            np.asarray(bo, np.float32).reshape(CT, 128).T,
        ],
        axis=1,
    )

    import ml_dtypes

    def pmajor(w, nt, dtype=np.float32):
        # [nt*128, cols] -> [128, nt*cols] with row p holding block-row t at
        # column block t (partition-major packing for a single DMA)
        return np.ascontiguousarray(
            np.asarray(w, np.float32)
            .reshape(nt, 128, -1)
            .transpose(1, 0, 2)
            .reshape(128, -1)
            .astype(dtype)
        )

    bf16 = ml_dtypes.bfloat16
    shared = {
        "wq": pmajor(Wq, CT),
        "wk": pmajor(Wk, KT, bf16),
        "wv": pmajor(Wv, KT, bf16),
        "wo": pmajor(Wo, PAIRS, bf16),
        "cst": np.ascontiguousarray(cst),
        "sel_bwd": sel_bwd,
        "stats_in": np.zeros((8, 8), np.float32),
        "stats_out": np.zeros((8, 8), np.float32),
    }

    in_maps = []
    for core in range(8):
        b, qt = core // 4, core % 4
        m = dict(shared)
        m["x_l"] = pmajor(x2[b, :, qt * NL:(qt + 1) * NL], CT)
        m["ctxT"] = pmajor(ctx[b].T, KT, bf16)
        in_maps.append(m)
    return in_maps


def _assemble(results):
    out = np.zeros((B, C, N), np.float32)
    for core in range(8):
        b, qt = core // 4, core % 4
        full = results[core]["out_l"].reshape(128, CT, NL)
        out[b, :, qt * NL:(qt + 1) * NL] = full.transpose(1, 0, 2).reshape(C, NL)
    return out.reshape(B, C, 16, 16, 16)


def run(inputs, trace=False):
    global _CACHED_NC
    if _CACHED_NC is None:
        _CACHED_NC = build_nc()
    nc = _CACHED_NC
    in_maps = _host_prep(**inputs)
    # stats_in/stats_out are internal dram tensors, not ExternalInputs
    for m in in_maps:
        m.pop("stats_in")
        m.pop("stats_out")
    bkr = bass_utils.run_bass_kernel_spmd(
        nc, in_maps, list(range(8)), trace=trace
    )
    return _assemble(bkr.results), bkr


def kernel(**inputs):
    out, _ = run(inputs)
    return out

